# revision 5
# baseline (speedup 1.0000x reference)
"""Trainium2 Bass kernel for PointProp message passing + FC head.

Reference computation (per point n, K=8 components, D=E=256, H=132):
    right = (sum_k comp_k) @ Wm.T + K*bm
    right = right @ Wu.T + bu
    x     = [signal, right]                       # [N, 512]
    h     = relu-MLP(x; W0,W1,W2) ; out = h @ W3.T + b3

Key algebraic fold (host-side, fp64): `right` only enters through W0's
right half W0r, so
    A  = W0r @ Wu @ Wm                            # [132, 256]
    c0 = W0r @ (K*(Wu@bm) + bu) + b0              # [132]
    h0 = relu(signal @ W0s.T + (sum_k comp_k) @ A.T + c0)
which removes both DxD linear layers from the device kernel.

Device kernel (per core, data-parallel over N):
  - stream 512-point superblocks
  - K-sum on DVE (in-place add tree on a k-interleaved tile)
  - PE-transpose signal/comp_sum to feature-major (PSUM), ACT copies out
  - FC stack as fp32r matmuls, H=132 split into 128+4 partitions
  - output stored feature-major [256, nsh]; host transposes back
"""

import numpy as np

import concourse.bacc as bacc
import concourse.bass as bass
import concourse.mybir as mybir
import concourse.tile as tile
from concourse.bass_utils import run_bass_kernel_spmd
from concourse.masks import make_identity

F32 = mybir.dt.float32
F32R = mybir.dt.float32r

N_FULL = 65536
K = 8
D = 256
E = 256
H = 132
NCORES = 8
NSH = N_FULL // NCORES  # 8192 points per core
SBLK = 512              # points per superblock
BLK4 = SBLK // 128      # 128-point blocks per superblock


def build_nc(nsh=NSH, use_f32r=True):
    """Build the single-core Bass program (same program runs SPMD on all cores)."""
    nblk = nsh // SBLK
    nc = bacc.Bacc("TRN2", target_bir_lowering=False, debug=False)

    comp = nc.declare_dram_parameter("comp", [K, nsh, D], F32, isOutput=False)
    sig = nc.declare_dram_parameter("sig", [nsh, E], F32, isOutput=False)
    w0 = nc.declare_dram_parameter("w0t", [E + D, H], F32R, isOutput=False)
    w1 = nc.declare_dram_parameter("w1t", [H, H], F32R, isOutput=False)
    w2 = nc.declare_dram_parameter("w2t", [H, H], F32R, isOutput=False)
    w3 = nc.declare_dram_parameter("w3t", [H, D], F32R, isOutput=False)
    c0 = nc.declare_dram_parameter("c0", [H, 1], F32, isOutput=False)
    c1 = nc.declare_dram_parameter("c1", [H, 1], F32, isOutput=False)
    c2 = nc.declare_dram_parameter("c2", [H, 1], F32, isOutput=False)
    c3 = nc.declare_dram_parameter("c3", [D, 1], F32, isOutput=False)
    outT = nc.declare_dram_parameter("outT", [D, nsh], F32, isOutput=True)

    # [s, k, p, b, d]: superblock, component, partition(point-in-block), block, feature
    comp_v = comp.ap().rearrange("k (s b p) d -> s k p b d", p=128, b=BLK4)
    sig_v = sig.ap().rearrange("(s b p) d -> s p b d", p=128, b=BLK4)
    outT_v = outT.ap()

    mmdt = F32R if use_f32r else F32

    def mm(out, lhsT, rhs, start, stop):
        nc.tensor.matmul(
            out, lhsT.bitcast(mmdt), rhs.bitcast(mmdt), start=start, stop=stop
        )

    with tile.TileContext(nc) as tc:
        with (
            tc.tile_pool(name="const", bufs=1) as cpool,
            tc.tile_pool(name="comp", bufs=2) as comp_pool,
            tc.tile_pool(name="sig", bufs=2) as sig_pool,
            tc.tile_pool(name="xt", bufs=2) as xt_pool,
            tc.tile_pool(name="hsb", bufs=2) as h_pool,
            tc.tile_pool(name="osb", bufs=3) as o_pool,
            tc.tile_pool(name="tpsum", bufs=2, space="PSUM") as tpsum,
            tc.tile_pool(name="hpsum", bufs=3, space="PSUM") as hpsum,
            tc.tile_pool(name="qpsum", bufs=2, space="PSUM") as qpsum,
        ):
            ident = cpool.tile([128, 128], F32)
            make_identity(nc, ident)

            w0t = cpool.tile([128, 4, H], F32R)
            nc.sync.dma_start(w0t, w0.ap().rearrange("(c p) m -> p c m", p=128))
            w1a = cpool.tile([128, H], F32R)
            nc.sync.dma_start(w1a, w1.ap()[0:128, :])
            w1b = cpool.tile([4, H], F32R)
            nc.sync.dma_start(w1b, w1.ap()[128:H, :])
            w2a = cpool.tile([128, H], F32R)
            nc.sync.dma_start(w2a, w2.ap()[0:128, :])
            w2b = cpool.tile([4, H], F32R)
            nc.sync.dma_start(w2b, w2.ap()[128:H, :])
            w3a = cpool.tile([128, D], F32R)
            nc.sync.dma_start(w3a, w3.ap()[0:128, :])
            w3b = cpool.tile([4, D], F32R)
            nc.sync.dma_start(w3b, w3.ap()[128:H, :])

            c0a = cpool.tile([128, 1], F32)
            nc.sync.dma_start(c0a, c0.ap()[0:128, :])
            c0b = cpool.tile([4, 1], F32)
            nc.sync.dma_start(c0b, c0.ap()[128:H, :])
            c1a = cpool.tile([128, 1], F32)
            nc.sync.dma_start(c1a, c1.ap()[0:128, :])
            c1b = cpool.tile([4, 1], F32)
            nc.sync.dma_start(c1b, c1.ap()[128:H, :])
            c2a = cpool.tile([128, 1], F32)
            nc.sync.dma_start(c2a, c2.ap()[0:128, :])
            c2b = cpool.tile([4, 1], F32)
            nc.sync.dma_start(c2b, c2.ap()[128:H, :])
            c3t = cpool.tile([128, 2], F32)
            nc.sync.dma_start(c3t, c3.ap().rearrange("(c p) o -> p (c o)", p=128))

            relu = mybir.ActivationFunctionType.Relu
            idf = mybir.ActivationFunctionType.Identity

            for s in range(nblk):
                # ---- load ----
                ct = comp_pool.tile([128, K, BLK4 * D], F32, tag="ct")
                for kk in range(K):
                    nc.sync.dma_start(ct[:, kk, :], comp_v[s, kk])
                st = sig_pool.tile([128, BLK4, E], F32, tag="st")
                nc.sync.dma_start(st, sig_v[s])

                # ---- K-sum (in-place tree on DVE) ----
                nc.vector.tensor_add(ct[:, 0:4, :], ct[:, 0:4, :], ct[:, 4:8, :])
                nc.vector.tensor_add(ct[:, 0:2, :], ct[:, 0:2, :], ct[:, 2:4, :])
                nc.vector.tensor_add(ct[:, 0, :], ct[:, 0, :], ct[:, 1, :])
                cs = ct[:, 0, :].rearrange("p (b d) -> p b d", b=BLK4)

                # ---- transpose to feature-major: xT[ch] = x^T chunk [128, 512] ----
                xT = []
                for ch in range(2):  # signal features
                    ps = tpsum.tile([128, SBLK], F32, tag="tp")
                    for b in range(BLK4):
                        nc.tensor.transpose(
                            ps[:, b * 128 : (b + 1) * 128],
                            st[:, b, ch * 128 : (ch + 1) * 128],
                            ident,
                        )
                    t = xt_pool.tile([128, SBLK], F32R, tag=f"xT{ch}")
                    nc.scalar.copy(t, ps)
                    xT.append(t)
                for ch in range(2):  # comp_sum features
                    ps = tpsum.tile([128, SBLK], F32, tag="tp")
                    for b in range(BLK4):
                        nc.tensor.transpose(
                            ps[:, b * 128 : (b + 1) * 128],
                            cs[:, b, ch * 128 : (ch + 1) * 128],
                            ident,
                        )
                    t = xt_pool.tile([128, SBLK], F32R, tag=f"xT{2 + ch}")
                    nc.scalar.copy(t, ps)
                    xT.append(t)

                # ---- layer 0: h0 = relu(W0cat^T.T @ xT + c0), H split 128+4 ----
                h0p = hpsum.tile([128, SBLK], F32, tag="hp")
                for ch in range(4):
                    mm(h0p, w0t[:, ch, 0:128], xT[ch], ch == 0, ch == 3)
                h0q = qpsum.tile([4, SBLK], F32, tag="hq")
                for ch in range(4):
                    mm(h0q, w0t[:, ch, 128:H], xT[ch], ch == 0, ch == 3)
                h0a = h_pool.tile([128, SBLK], F32R, tag="h0a")
                nc.scalar.activation(h0a, h0p, relu, bias=c0a)
                h0b = h_pool.tile([4, SBLK], F32R, tag="h0b")
                nc.scalar.activation(h0b, h0q, relu, bias=c0b)

                # ---- layer 1 ----
                h1p = hpsum.tile([128, SBLK], F32, tag="hp")
                mm(h1p, w1a[:, 0:128], h0a, True, False)
                mm(h1p, w1b[:, 0:128], h0b, False, True)
                h1q = qpsum.tile([4, SBLK], F32, tag="hq")
                mm(h1q, w1a[:, 128:H], h0a, True, False)
                mm(h1q, w1b[:, 128:H], h0b, False, True)
                h1a = h_pool.tile([128, SBLK], F32R, tag="h1a")
                nc.scalar.activation(h1a, h1p, relu, bias=c1a)
                h1b = h_pool.tile([4, SBLK], F32R, tag="h1b")
                nc.scalar.activation(h1b, h1q, relu, bias=c1b)

                # ---- layer 2 ----
                h2p = hpsum.tile([128, SBLK], F32, tag="hp")
                mm(h2p, w2a[:, 0:128], h1a, True, False)
                mm(h2p, w2b[:, 0:128], h1b, False, True)
                h2q = qpsum.tile([4, SBLK], F32, tag="hq")
                mm(h2q, w2a[:, 128:H], h1a, True, False)
                mm(h2q, w2b[:, 128:H], h1b, False, True)
                h2a = h_pool.tile([128, SBLK], F32R, tag="h2a")
                nc.scalar.activation(h2a, h2p, relu, bias=c2a)
                h2b = h_pool.tile([4, SBLK], F32R, tag="h2b")
                nc.scalar.activation(h2b, h2q, relu, bias=c2b)

                # ---- layer 3: out^T = W3^T.T @ h2 + b3, D=256 in two 128 halves ----
                for half in range(2):
                    op = hpsum.tile([128, SBLK], F32, tag="hp")
                    mm(op, w3a[:, half * 128 : (half + 1) * 128], h2a, True, False)
                    mm(op, w3b[:, half * 128 : (half + 1) * 128], h2b, False, True)
                    ot = o_pool.tile([128, SBLK], F32, tag=f"ot{half}")
                    nc.scalar.activation(ot, op, idf, bias=c3t[:, half : half + 1])
                    nc.sync.dma_start(
                        outT_v[half * 128 : (half + 1) * 128, s * SBLK : (s + 1) * SBLK],
                        ot,
                    )

    nc.compile()
    return nc


def fold_weights(Wm, bm, Wu, bu, W0, b0, W1, b1, W2, b2, W3, b3, k):
    f8 = np.float64
    W0s = W0[:, :E].astype(f8)
    W0r = W0[:, E:].astype(f8)
    A = W0r @ Wu.astype(f8) @ Wm.astype(f8)
    c0 = W0r @ (k * (Wu.astype(f8) @ bm.astype(f8)) + bu.astype(f8)) + b0.astype(f8)
    w0t = np.ascontiguousarray(
        np.concatenate([W0s, A], axis=1).T.astype(np.float32)
    )  # [E+D, H]
    return {
        "w0t": w0t,
        "w1t": np.ascontiguousarray(W1.T.astype(np.float32)),
        "w2t": np.ascontiguousarray(W2.T.astype(np.float32)),
        "w3t": np.ascontiguousarray(W3.T.astype(np.float32)),
        "c0": np.ascontiguousarray(c0.astype(np.float32)[:, None]),
        "c1": np.ascontiguousarray(b1.astype(np.float32)[:, None]),
        "c2": np.ascontiguousarray(b2.astype(np.float32)[:, None]),
        "c3": np.ascontiguousarray(b3.astype(np.float32)[:, None]),
    }


_NC_CACHE = {}


def _get_nc(nsh=NSH):
    if nsh not in _NC_CACHE:
        _NC_CACHE[nsh] = build_nc(nsh)
    return _NC_CACHE[nsh]


def kernel(signal, components, Wm, bm, Wu, bu, W0, b0, W1, b1, W2, b2, W3, b3):
    signal = np.asarray(signal, dtype=np.float32)
    components = np.asarray(components, dtype=np.float32)
    wmap = fold_weights(Wm, bm, Wu, bu, W0, b0, W1, b1, W2, b2, W3, b3, K)

    nc = _get_nc()
    in_maps = []
    for i in range(NCORES):
        lo, hi = i * NSH, (i + 1) * NSH
        m = dict(wmap)
        m["comp"] = np.ascontiguousarray(components[:, lo:hi, :])
        m["sig"] = np.ascontiguousarray(signal[lo:hi, :])
        in_maps.append(m)

    res = run_bass_kernel_spmd(nc, in_maps, core_ids=list(range(NCORES)))
    out = np.concatenate(
        [np.asarray(r["outT"]).T for r in res.results], axis=0
    )
    return np.ascontiguousarray(out.astype(np.float32))
